# revision 45
# baseline (speedup 1.0000x reference)
"""Trainium2 Bass kernel for nn_HRRAdaptedAttention (B=2, S=8192, D=1024).

out = output + gate * irfft(cumsum_s(rfft(k)*rfft(v)) * conj(rfft(q))),
q/k/v = hidden @ W.T + b.

Single merged launch per core (chunk = 2048 positions, (batch, quarter)
per core). The rfft/irfft are folded into the projection weights on the
host; the nyquist row is packed into the (all-zero) sin(f=0) column of
each S-matrix, with 1-row DVE fixups where the complex-multiply formula
would mix the DC and nyquist rows. Projection matmuls run as fp8e4m3
hi+lo splits in DoubleRow perf mode (2 contraction slices/instr at 0.5
cyc/row); the cross-chunk cumsum carry is resolved in-kernel with an
AllGather of per-chunk totals (hidden under the fq matmuls), so there
is no second launch and no kv/ht DRAM round trip.
"""

import numpy as np
import ml_dtypes

B, S, D = 2, 8192, 1024
NCORES = 8
CHUNK = 2048
PANEL = 512
NPANEL = CHUNK // PANEL
FT = 4                    # 128-row freq tiles f=0..511 (nyq packed in im row 0)
NDP = 8                   # 128-row contraction slices of D
NPAIR = NDP // 2

F8NP = ml_dtypes.float8_e4m3
BF16NP = ml_dtypes.bfloat16

_cache = {}


def _split8(x):
    hi = x.astype(F8NP)
    lo = (x - hi.astype(np.float32)).astype(F8NP)
    return hi, lo


def _pack_w(M):
    """[1024, 512] f32 -> [128, dp(8), hl(2), 512] fp8 -> [128, 8192]."""
    hi, lo = _split8(np.asarray(M, np.float32))
    out = np.empty((128, NDP, 2, 512), F8NP)
    out[:, :, 0, :] = hi.reshape(NDP, 128, 512).transpose(1, 0, 2)
    out[:, :, 1, :] = lo.reshape(NDP, 128, 512).transpose(1, 0, 2)
    return out.reshape(128, -1)


def _pack_ht(ht):
    """[1024, 2048] f32 -> [128, pan(4), dp(8), hl(2), pos(512)] fp8."""
    hi, lo = _split8(ht)
    out = np.empty((128, NPANEL, NDP, 2, PANEL), F8NP)
    h4 = hi.reshape(NDP, 128, NPANEL, PANEL)
    l4 = lo.reshape(NDP, 128, NPANEL, PANEL)
    out[:, :, :, 0, :] = h4.transpose(1, 2, 0, 3)
    out[:, :, :, 1, :] = l4.transpose(1, 2, 0, 3)
    return out.reshape(128, -1)


def _host_constants(Wq, bq, Wk, bk, Wv, bv, gate):
    d = np.arange(D, dtype=np.float64)
    f = np.arange(513, dtype=np.float64)
    ang = 2.0 * np.pi * np.outer(d, f) / D
    C = np.cos(ang)
    Sm = -np.sin(ang)

    def fold(W, sign_s=1.0):
        Wt = W.T.astype(np.float64)
        MC = Wt @ C
        MS = sign_s * (Wt @ Sm)
        MS[:, 0] = MC[:, 512]          # nyquist packed into sin(f=0) col
        return MC[:, :512].astype(np.float32), MS[:, :512].astype(np.float32)

    MkC, MkS = fold(Wk)
    MvC, MvS = fold(Wv)
    MqC, MqS = fold(Wq, sign_s=-1.0)   # conj(fq) folded; nyq col has no sign
    # Z = mem*conj(fq) is ~1e6-scale, far outside fp8 range. The fq
    # PSUM->SBUF copy applies 2^-19 (activation scale) so Z comes out
    # pre-scaled; A/B carry the inverse. Pure exponent shift.
    ZSC = 2.0 ** -19

    g = float(np.asarray(gate).reshape(-1)[0])
    w = np.full(513, 2.0)
    w[0] = 1.0
    w[512] = 1.0
    scale = (w * g / (D * ZSC))[:, None]
    A = (scale * C.T).astype(np.float32)       # [513, D]
    Bm = (scale * Sm.T).astype(np.float32)
    # Conjugate symmetry of the inverse transform: vals[512+j] =
    # U[512-j] - V[512-j] (+ even nyquist term), so the matmuls only
    # need columns d=0..512. Row layout: rows 0..511 = A (re rows,
    # cos, even); rows 512..1023 = B (sin rows, odd) with the f=0 slot
    # zeroed; row 1024 = the nyquist row (cos-type, even — it rides
    # with U via a K=1 matmul from its own partition-0 tile).
    Bout = Bm[:512, :513].copy()
    Bout[0] = 0.0
    Aout = A[:512, :513]
    nyq = A[512:513, :513]

    def foldb(b, sign_s=1.0):
        b64 = np.asarray(b, np.float64)
        bc = b64 @ C
        bs = sign_s * (b64 @ Sm)
        bs[0] = bc[512]
        return bc[:512].astype(np.float32), bs[:512].astype(np.float32)

    bkC, bkS = foldb(bk)
    bvC, bvS = foldb(bv)
    bqC, bqS = foldb(bq, sign_s=-1.0)
    bqC *= ZSC
    bqS *= ZSC
    biasc = np.stack([bkC, bkS, bvC, bvS, bqC, bqS])   # [6, 512]

    wall = np.concatenate(
        [_pack_w(M) for M in (MkC, MkS, MvC, MvS, MqC, MqS)], axis=1)
    ab = np.concatenate([Aout, Bout, nyq], axis=0).astype(BF16NP)  # [1025, 513]
    return dict(wall=wall, ab=ab, biasc=biasc)


_WAIT_EXEMPT = {
    "InstNoOp", "InstEventSemaphore", "InstUnconditionalBranch",
    "InstRegisterMove", "InstCall", "InstISA",
}


def _legalize_waits(nc, max_waits=1):
    """TRN2 instruction structs hold one sync-wait command; move extra waits
    onto same-engine nops inserted just before the instruction."""
    import bass_rust
    import concourse.mybir as mybir
    ctr = 0
    for fn in nc.m.functions:
        for blk in fn.blocks:
            new = []
            for inst in blk.instructions:
                if (type(inst).__name__ not in _WAIT_EXEMPT
                        and inst.sync_info is not None):
                    waits = list(inst.sync_info.on_wait)
                    if len(waits) > max_waits:
                        for w in waits[:-max_waits]:
                            nop = mybir.InstNoOp(
                                name=f"I-lglnop-{ctr}", ins=[], outs=[])
                            ctr += 1
                            nop.engine = inst.engine
                            nop.sync_info = bass_rust.SyncInfo(
                                on_wait=[w], on_update=[])
                            new.append(nop)
                        inst.sync_info = bass_rust.SyncInfo(
                            on_wait=waits[-max_waits:],
                            on_update=inst.sync_info.on_update)
                new.append(inst)
            blk.instructions = new


def _build(has_bias):
    import concourse.bass as bass
    import concourse.mybir as mybir
    import concourse.tile as tile
    F32, F8, BF16 = mybir.dt.float32, mybir.dt.float8e4, mybir.dt.bfloat16
    AT = mybir.AluOpType
    DR = mybir.MatmulPerfMode.DoubleRow

    nc = bass.Bass("TRN2", target_bir_lowering=False, debug=False,
                   num_devices=NCORES)
    htp_d = nc.dram_tensor("htp", [128, NPANEL * 8192], F8,
                           kind="ExternalInput")
    w_d = nc.dram_tensor("wall", [128, 6 * 8192], F8, kind="ExternalInput")
    ab_d = nc.dram_tensor("ab", [1025, 513], BF16, kind="ExternalInput")
    outp_d = nc.dram_tensor("outp", [CHUNK, D], BF16, kind="ExternalInput")
    mask_d = nc.dram_tensor("mask", [128, 64], F32, kind="ExternalInput")
    if has_bias:
        biasc_d = nc.dram_tensor("biasc", [128, 24], F32,
                                 kind="ExternalInput")
    res_d = nc.dram_tensor("res", [CHUNK, D], BF16, kind="ExternalOutput")
    cc_in = nc.dram_tensor("cc_in", [128, 8], F32)
    cc_out = nc.dram_tensor("cc_out", [NCORES * 128, 8], F32)

    with tile.TileContext(nc) as tc:
        with (
            tc.tile_pool(name="wpool", bufs=1) as wp,
            tc.tile_pool(name="htpool", bufs=1) as hp,
            tc.tile_pool(name="const", bufs=1) as cp,
            tc.tile_pool(name="fkv", bufs=2) as fkp,
            tc.tile_pool(name="fq3", bufs=4) as fqp,
            tc.tile_pool(name="tt", bufs=6) as ttp,
            tc.tile_pool(name="mem", bufs=1) as memp,
            tc.tile_pool(name="z", bufs=2) as zp,
            tc.tile_pool(name="io", bufs=6) as iop,
            tc.tile_pool(name="sv", bufs=5) as svp,
            tc.tile_pool(name="ps", bufs=2, space="PSUM") as psp,
        ):
            # DMA order is tuned so the PE can start ~7us in: hi halves of
            # the k/v weights and panel-0 ht first, then the lo halves.
            w = [wp.tile([128, 8192], F8, tag=f"w{m}", name=f"w{m}") for m in range(6)]
            htp = [hp.tile([128, 8192], F8, tag=f"ht{p}", name=f"ht{p}")
                   for p in range(NPANEL)]
            wvd = w_d.ap().rearrange("p (m dp hl f) -> p m dp hl f",
                                     m=6, dp=NDP, hl=2)
            htd = htp_d.ap().rearrange("p (pan dp hl x) -> p pan dp hl x",
                                       pan=NPANEL, dp=NDP, hl=2)
            wvs = [w[m][:].rearrange("p (dp hl f) -> p dp hl f",
                                     dp=NDP, hl=2) for m in range(6)]
            hts = [htp[p][:].rearrange("p (dp hl x) -> p dp hl x",
                                       dp=NDP, hl=2) for p in range(NPANEL)]

            def load_w(m, hl, pairs=((0, 8),)):
                for (a, b) in pairs:
                    nc.sync.dma_start(wvs[m][:, a:b, hl, :],
                                      wvd[:, m, a:b, hl, :])

            def load_ht(p, hl, pairs=((0, 8),)):
                for (a, b) in pairs:
                    nc.sync.dma_start(hts[p][:, a:b, hl, :],
                                      htd[:, p, a:b, hl, :])

            # first weight/data slices arrive pair-granular and interleaved
            # so the first matmul can start after two small transfers
            for (a, b) in ((0, 2), (2, 4), (4, 6), (6, 8)):
                load_ht(0, 0, ((a, b),))
                load_w(0, 0, ((a, b),))
            load_w(0, 1, ((0, 2), (2, 4), (4, 6), (6, 8)))
            load_ht(0, 1)
            for m in range(1, 4):
                load_w(m, 0)
                load_w(m, 1)
            load_w(4, 0)
            load_w(5, 0)
            load_w(4, 1)
            load_w(5, 1)
            for p in range(1, NPANEL):
                load_ht(p, 0)
                load_ht(p, 1)
            ab = [wp.tile([128, 513], BF16, tag=f"ab{i}", name=f"ab{i}")
                  for i in range(8)]
            for i in range(8):
                nc.sync.dma_start(ab[i][:], ab_d.ap()[i * 128:(i + 1) * 128, :])
            bny = wp.tile([1, 513], BF16, tag="bny", name="bny")
            nc.sync.dma_start(bny[:], ab_d.ap()[1024:1025, :])
            mask = cp.tile([128, 64], F32, tag="mask", name="mask")
            nc.sync.dma_start(mask[:], mask_d.ap())
            # preload the residual into the output buffer (DRAM->DRAM) on
            # the same software-DGE queue the tail accumulates use, so the
            # final per-sub DMA is a single accumulate instead of two hops
            for r0 in range(0, CHUNK, PANEL):
                nc.gpsimd.dma_start(res_d.ap()[r0:r0 + PANEL, :],
                                    outp_d.ap()[r0:r0 + PANEL, :])
            if has_bias:
                biasc = cp.tile([128, 24], F32, tag="biasc", name="biasc")
                nc.sync.dma_start(biasc[:], biasc_d.ap())

            wv, htv = wvs, hts

            def fwd_matmuls(pt, m, pan, ft, combos=3):
                # hh first (hi weights + hi data), then lh (lo weights),
                # then hl (lo data) — matches the DMA arrival order.
                # k/v (m<4) run 2 combos (full-W x hi-X): the dropped W@Xlo
                # term costs ~1e-2 rel err, inside the 2e-2 budget; q keeps
                # all 3 (its error hits Z unaveraged).
                ii = 0
                for (whl, xhl) in ((0, 0), (1, 0), (0, 1))[:combos]:
                    for a in range(NPAIR):
                        nc.tensor.matmul(
                            pt[:],
                            wv[m][:, 2 * a:2 * a + 2, whl,
                                  ft * 128:(ft + 1) * 128],
                            htv[pan][:, 2 * a:2 * a + 2, xhl, :],
                            start=(ii == 0), stop=(ii == combos * NPAIR - 1),
                            perf_mode=DR)
                        ii += 1

            mem = {}
            # ---- loop A: fk, fv, kv, local scan --------------------------
            for pan in range(NPANEL):
                for ft in range(FT):
                    ps = {}
                    for m, nm in enumerate(("kre", "kim", "vre", "vim")):
                        pt = psp.tile([128, PANEL], F32, tag=f"ps_{nm}", name=f"ps_{nm}")
                        fwd_matmuls(pt, m, pan, ft, combos=2)
                        ps[nm] = pt
                    s = {}
                    for m, nm in enumerate(("kre", "kim", "vre", "vim")):
                        t = fkp.tile([128, PANEL], BF16, tag=f"s_{nm}", name=f"s_{nm}")
                        nc.scalar.copy(t[:], ps[nm][:])
                        if has_bias:
                            c = m * 4 + ft
                            nc.vector.tensor_scalar_add(
                                t[:], t[:], biasc[:, c:c + 1])
                        s[nm] = t
                    # complex product folded into the scans:
                    # re: state = (state + t1) - t2, im: state = (state + t3) + t4
                    t1 = ttp.tile([128, PANEL], BF16, tag="tt", name="tt")
                    nc.vector.tensor_tensor(t1[:], s["kre"][:], s["vre"][:],
                                            op=AT.mult)
                    t2 = ttp.tile([128, PANEL], BF16, tag="tt", name="tt")
                    nc.vector.tensor_tensor(t2[:], s["kim"][:], s["vim"][:],
                                            op=AT.mult)
                    t3 = ttp.tile([128, PANEL], BF16, tag="tt", name="tt")
                    nc.vector.tensor_tensor(t3[:], s["kre"][:], s["vim"][:],
                                            op=AT.mult)
                    t4 = ttp.tile([128, PANEL], BF16, tag="tt", name="tt")
                    nc.vector.tensor_tensor(t4[:], s["kim"][:], s["vre"][:],
                                            op=AT.mult)

                    def init_of(ri, rows=slice(None)):
                        if pan == 0:
                            return 0.0
                        return mem[(pan - 1, ri, ft)][rows, PANEL - 1:PANEL]

                    mre = memp.tile([128, PANEL], BF16,
                                    tag=f"mem_re{ft}_{pan}",
                                    name=f"mem_re{ft}_{pan}")
                    nc.vector.tensor_tensor_scan(
                        mre[:], t1[:], t2[:], init_of("re"),
                        op0=AT.add, op1=AT.subtract)
                    mim = memp.tile([128, PANEL], BF16,
                                    tag=f"mem_im{ft}_{pan}",
                                    name=f"mem_im{ft}_{pan}")
                    nc.vector.tensor_tensor_scan(
                        mim[:], t3[:], t4[:], init_of("im"),
                        op0=AT.add, op1=AT.add)
                    if ft == 0:
                        # row 0 carries (DC, nyquist): pure real products
                        # (re += t1 only, im += t2 only); rescan that row.
                        nc.vector.tensor_tensor_scan(
                            mre[0:1, :], t1[0:1, :], t1[0:1, :],
                            init_of("re", slice(0, 1)),
                            op0=AT.add, op1=AT.bypass)
                        nc.vector.tensor_tensor_scan(
                            mim[0:1, :], t2[0:1, :], t2[0:1, :],
                            init_of("im", slice(0, 1)),
                            op0=AT.add, op1=AT.bypass)
                    mem[(pan, "re", ft)] = mre
                    mem[(pan, "im", ft)] = mim

            # ---- totals exchange (hidden under fq matmuls) ---------------
            tot = cp.tile([128, 8], F32, tag="tot", name="tot")
            for ft in range(FT):
                nc.gpsimd.tensor_copy(
                    tot[:, ft:ft + 1],
                    mem[(NPANEL - 1, "re", ft)][:, PANEL - 1:PANEL])
                nc.gpsimd.tensor_copy(
                    tot[:, 4 + ft:5 + ft],
                    mem[(NPANEL - 1, "im", ft)][:, PANEL - 1:PANEL])
            nc.sync.dma_start(cc_in.ap(), tot[:])
            nc.gpsimd.collective_compute(
                "AllGather", AT.bypass,
                replica_groups=[list(range(NCORES))],
                ins=[cc_in[:].opt()], outs=[cc_out[:].opt()])
            g = cp.tile([128, 64], F32, tag="g", name="g")
            nc.sync.dma_start(
                g[:].rearrange("p (c j) -> p c j", c=NCORES),
                cc_out.ap().rearrange("(c p) j -> p c j", c=NCORES))
            gm = cp.tile([128, 64], F32, tag="gm", name="gm")
            nc.vector.tensor_tensor(gm[:], g[:], mask[:], op=AT.mult)
            pref = cp.tile([128, 8], F32, tag="pref", name="pref")
            nc.vector.tensor_reduce(
                pref[:], gm[:].rearrange("p (c j) -> p j c", c=8),
                axis=mybir.AxisListType.X, op=AT.add)

            # ---- loop C: fq, prefix, Z, output matmul, residual ----------
            # fq blocks run two panels ahead of the Z/output blocks so the
            # PE keeps streaming fq matmuls while the AllGather completes.
            def fq_block(pan):
                sq = {}
                for ft in range(FT):
                    for m, nm in ((4, "qre"), (5, "qim")):
                        pt = psp.tile([128, PANEL], F32,
                                      tag=("ps_kre" if nm == "qre"
                                           else "ps_kim"),
                                      name=f"ps_{nm}")
                        fwd_matmuls(pt, m, pan, ft)
                        t = fqp.tile([128, PANEL], BF16, tag=f"s_{nm}{ft}",
                                     name=f"s_{nm}{ft}")
                        nc.scalar.activation(
                            t[:], pt[:], mybir.ActivationFunctionType.Copy,
                            scale=2.0 ** -19)
                        if has_bias:
                            c = m * 4 + ft
                            nc.vector.tensor_scalar_add(
                                t[:], t[:], biasc[:, c:c + 1])
                        sq[(nm, ft)] = t
                return sq

            def zprod_block(pan, sq):
                z = {}
                for ft in range(FT):
                    mre = mem[(pan, "re", ft)]
                    mim = mem[(pan, "im", ft)]
                    nc.vector.tensor_scalar_add(mre[:], mre[:],
                                                pref[:, ft:ft + 1])
                    nc.vector.tensor_scalar_add(mim[:], mim[:],
                                                pref[:, 4 + ft:5 + ft])
                    sqre, sqim = sq[("qre", ft)], sq[("qim", ft)]
                    for ri, (a, b_) in (("re", (sqre, sqim)),
                                        ("im", (sqim, sqre))):
                        neg = ri == "re"
                        t1 = ttp.tile([128, PANEL], BF16, tag="tt", name="tt")
                        nc.vector.tensor_tensor(t1[:], mre[:], a[:],
                                                op=AT.mult)
                        t2 = ttp.tile([128, PANEL], BF16, tag="tt", name="tt")
                        eng_m = (nc.gpsimd if ri == "im" and ft % 2 == 0
                                 else nc.vector)
                        eng_m.tensor_tensor(t2[:], mim[:], b_[:],
                                            op=AT.mult)
                        zt = zp.tile([128, PANEL], BF16, tag=f"z_{ri}{ft}",
                                     name=f"z_{ri}{ft}")
                        eng = nc.gpsimd if neg else nc.vector
                        eng.tensor_tensor(
                            zt[:], t1[:], t2[:],
                            op=(AT.subtract if neg else AT.add))
                        if ft == 0:
                            # row 0 carries (DC, nyq): plain real products
                            nc.vector.tensor_tensor(
                                zt[0:1, :], (mre if neg else mim)[0:1, :],
                                (sqre if neg else sqim)[0:1, :], op=AT.mult)
                        z[(ri, ft)] = zt
                return z

            def zout_mm(pan, z):
                for sub in range(4):
                    r0 = pan * PANEL + sub * 128
                    rs = iop.tile([128, D], BF16, tag="rs", name="rs")
                    s0, s1c = sub * 128, (sub + 1) * 128
                    # U = sum_ft zre.A1 (+ nyquist row, even, via K=1),
                    # V = sum zim.B1 (sin rows only). vals[0:512] = U + V,
                    # vals[512+j] = U[512-j] - V[512-j]; the d=512 edge
                    # column accumulates into V's (all-zero) column 0.
                    # Rotate over all four PSUM tag pairs (the fq tags are
                    # free once the zout phase runs) so four subs can be in
                    # flight before a combine has to retire.
                    tU, tV = (("ps_vre", "ps_vim") if sub % 2 == 0
                              else ("ps_kre", "ps_kim"))
                    psU = psp.tile([128, 512], F32, tag=tU, name="ps_U")
                    psV = psp.tile([128, 512], F32, tag=tV, name="ps_V")
                    for ft in range(FT):
                        nc.tensor.matmul(
                            psU[:], z[("re", ft)][:, s0:s1c],
                            ab[ft][:, 0:512], start=(ft == 0), stop=False)
                    nc.tensor.matmul(
                        psU[:], z[("im", 0)][0:1, s0:s1c],
                        bny[0:1, 0:512], start=False, stop=True)
                    for ft in range(FT):
                        nc.tensor.matmul(
                            psV[:], z[("im", ft)][:, s0:s1c],
                            ab[4 + ft][:, 0:512],
                            start=(ft == 0), stop=(ft == FT - 1))
                    for ft in range(FT):
                        nc.tensor.matmul(
                            psV[:, 0:1], z[("re", ft)][:, s0:s1c],
                            ab[ft][:, 512:513], start=(ft == 0), stop=False)
                    nc.tensor.matmul(
                        psV[:, 0:1], z[("im", 0)][0:1, s0:s1c],
                        bny[0:1, 512:513], start=False, stop=True)
                    # bf16 combine: both PSUM halves copied to SBUF by the
                    # Act engine so the DVE ops run at the 2-byte rate and
                    # the residual/output DMAs halve their traffic. Each
                    # 512-column half ships as soon as it is complete.
                    sU = svp.tile([128, 512], BF16, tag="sU", name="sU")
                    nc.scalar.copy(sU[:], psU[:])
                    sV = svp.tile([128, 512], BF16, tag="sV", name="sV")
                    nc.scalar.copy(sV[:], psV[:])
                    nc.vector.tensor_copy(rs[:, 0:1], sU[:, 0:1])
                    nc.vector.tensor_copy(rs[:, 512:513], sV[:, 0:1])
                    nc.vector.tensor_tensor(rs[:, 513:1024],
                                            sU[:, 511:0:-1],
                                            sV[:, 511:0:-1],
                                            op=AT.subtract)
                    nc.vector.tensor_tensor(rs[:, 1:512], sU[:, 1:512],
                                            sV[:, 1:512], op=AT.add)
                    # single accumulate-out into the preloaded res buffer
                    nc.gpsimd.dma_start(res_d.ap()[r0:r0 + 128, :], rs[:],
                                        accum_op=AT.add)

            # all fq blocks run first: ~40us of PE work that fully hides the
            # AllGather + prefix chain; z-product blocks stay one panel ahead
            # of the output matmuls so the PE never waits on the DVE.
            sqs = {p: fq_block(p) for p in range(NPANEL)}
            zps = {0: zprod_block(0, sqs[0]), 1: zprod_block(1, sqs[1])}
            for pan in range(NPANEL):
                if pan + 2 < NPANEL:
                    zps[pan + 2] = zprod_block(pan + 2, sqs[pan + 2])
                zout_mm(pan, zps.pop(pan))

    _legalize_waits(nc)
    return nc


def _program(has_bias=False):
    key = ("merged", has_bias)
    if key not in _cache:
        _cache[key] = _build(has_bias)
    return _cache[key]


def kernel(output, hidden_states, Wq, bq, Wk, bk, Wv, bv, gate, _trace=False):
    from concourse import bass_utils

    output = np.asarray(output, dtype=np.float32)
    hidden = np.asarray(hidden_states, dtype=np.float32)
    cst = _host_constants(
        np.asarray(Wq, np.float32), np.asarray(bq, np.float32),
        np.asarray(Wk, np.float32), np.asarray(bk, np.float32),
        np.asarray(Wv, np.float32), np.asarray(bv, np.float32),
        np.asarray(gate, np.float32))
    has_bias = bool(np.any(cst["biasc"]))
    nc = _program(has_bias)

    chunks = [(c // 4, c % 4) for c in range(NCORES)]
    shared = {"wall": cst["wall"], "ab": cst["ab"]}
    if has_bias:
        bc = np.zeros((128, 24), np.float32)
        for m in range(6):
            bc[:, m * 4:(m + 1) * 4] = cst["biasc"][m].reshape(4, 128).T
        shared["biasc"] = bc

    in_maps = []
    for c, (b, j) in enumerate(chunks):
        im = dict(shared)
        ht = np.ascontiguousarray(
            hidden[b, j * CHUNK:(j + 1) * CHUNK, :].T)
        im["htp"] = _pack_ht(ht)
        im["outp"] = output[b, j * CHUNK:(j + 1) * CHUNK, :].astype(BF16NP)
        mask = np.zeros((128, 64), np.float32)
        for c2, (b2, j2) in enumerate(chunks):
            if b2 == b and j2 < j:
                mask[:, c2 * 8:(c2 + 1) * 8] = 1.0
        im["mask"] = mask
        in_maps.append(im)

    res = bass_utils.run_bass_kernel_spmd(
        nc, in_maps, core_ids=list(range(NCORES)), trace=_trace)

    out = np.empty((B, S, D), dtype=np.float32)
    for c, (b, j) in enumerate(chunks):
        out[b, j * CHUNK:(j + 1) * CHUNK, :] = res.results[c]["res"].astype(
            np.float32)
    if _trace:
        kernel._last = res
    return out



# revision 47
# speedup vs baseline: 1.0181x; 1.0181x over previous
"""Trainium2 Bass kernel for nn_HRRAdaptedAttention (B=2, S=8192, D=1024).

out = output + gate * irfft(cumsum_s(rfft(k)*rfft(v)) * conj(rfft(q))),
q/k/v = hidden @ W.T + b.

Single merged launch per core (chunk = 2048 positions, (batch, quarter)
per core). The rfft/irfft are folded into the projection weights on the
host; the nyquist row is packed into the (all-zero) sin(f=0) column of
each S-matrix, with 1-row DVE fixups where the complex-multiply formula
would mix the DC and nyquist rows. Projection matmuls run as fp8e4m3
hi+lo splits in DoubleRow perf mode (2 contraction slices/instr at 0.5
cyc/row); the cross-chunk cumsum carry is resolved in-kernel with an
AllGather of per-chunk totals (hidden under the fq matmuls), so there
is no second launch and no kv/ht DRAM round trip.
"""

import numpy as np
import ml_dtypes

B, S, D = 2, 8192, 1024
NCORES = 8
CHUNK = 2048
PANEL = 512
NPANEL = CHUNK // PANEL
FT = 4                    # 128-row freq tiles f=0..511 (nyq packed in im row 0)
NDP = 8                   # 128-row contraction slices of D
NPAIR = NDP // 2

F8NP = ml_dtypes.float8_e4m3
BF16NP = ml_dtypes.bfloat16

_cache = {}


def _split8(x):
    hi = x.astype(F8NP)
    lo = (x - hi.astype(np.float32)).astype(F8NP)
    return hi, lo


def _pack_w(M):
    """[1024, 512] f32 -> [128, dp(8), hl(2), 512] fp8 -> [128, 8192]."""
    hi, lo = _split8(np.asarray(M, np.float32))
    out = np.empty((128, NDP, 2, 512), F8NP)
    out[:, :, 0, :] = hi.reshape(NDP, 128, 512).transpose(1, 0, 2)
    out[:, :, 1, :] = lo.reshape(NDP, 128, 512).transpose(1, 0, 2)
    return out.reshape(128, -1)


def _pack_ht(ht):
    """[1024, 2048] f32 -> [128, pan(4), dp(8), hl(2), pos(512)] fp8."""
    hi, lo = _split8(ht)
    out = np.empty((128, NPANEL, NDP, 2, PANEL), F8NP)
    h4 = hi.reshape(NDP, 128, NPANEL, PANEL)
    l4 = lo.reshape(NDP, 128, NPANEL, PANEL)
    out[:, :, :, 0, :] = h4.transpose(1, 2, 0, 3)
    out[:, :, :, 1, :] = l4.transpose(1, 2, 0, 3)
    return out.reshape(128, -1)


def _host_constants(Wq, bq, Wk, bk, Wv, bv, gate):
    d = np.arange(D, dtype=np.float64)
    f = np.arange(513, dtype=np.float64)
    ang = 2.0 * np.pi * np.outer(d, f) / D
    C = np.cos(ang)
    Sm = -np.sin(ang)

    def fold(W, sign_s=1.0):
        Wt = W.T.astype(np.float64)
        MC = Wt @ C
        MS = sign_s * (Wt @ Sm)
        MS[:, 0] = MC[:, 512]          # nyquist packed into sin(f=0) col
        return MC[:, :512].astype(np.float32), MS[:, :512].astype(np.float32)

    MkC, MkS = fold(Wk)
    MvC, MvS = fold(Wv)
    MqC, MqS = fold(Wq, sign_s=-1.0)   # conj(fq) folded; nyq col has no sign
    # Z = mem*conj(fq) is ~1e6-scale, far outside fp8 range. The fq
    # PSUM->SBUF copy applies 2^-19 (activation scale) so Z comes out
    # pre-scaled; A/B carry the inverse. Pure exponent shift.
    ZSC = 2.0 ** -19

    g = float(np.asarray(gate).reshape(-1)[0])
    w = np.full(513, 2.0)
    w[0] = 1.0
    w[512] = 1.0
    scale = (w * g / (D * ZSC))[:, None]
    A = (scale * C.T).astype(np.float32)       # [513, D]
    Bm = (scale * Sm.T).astype(np.float32)
    # Conjugate symmetry of the inverse transform: vals[512+j] =
    # U[512-j] - V[512-j] (+ even nyquist term), so the matmuls only
    # need columns d=0..512. Row layout: rows 0..511 = A (re rows,
    # cos, even); rows 512..1023 = B (sin rows, odd) with the f=0 slot
    # zeroed; row 1024 = the nyquist row (cos-type, even — it rides
    # with U via a K=1 matmul from its own partition-0 tile).
    Bout = Bm[:512, :513].copy()
    Bout[0] = 0.0
    Aout = A[:512, :513]
    nyq = A[512:513, :513]

    def foldb(b, sign_s=1.0):
        b64 = np.asarray(b, np.float64)
        bc = b64 @ C
        bs = sign_s * (b64 @ Sm)
        bs[0] = bc[512]
        return bc[:512].astype(np.float32), bs[:512].astype(np.float32)

    bkC, bkS = foldb(bk)
    bvC, bvS = foldb(bv)
    bqC, bqS = foldb(bq, sign_s=-1.0)
    bqC *= ZSC
    bqS *= ZSC
    biasc = np.stack([bkC, bkS, bvC, bvS, bqC, bqS])   # [6, 512]

    wall = np.concatenate(
        [_pack_w(M) for M in (MkC, MkS, MvC, MvS, MqC, MqS)], axis=1)
    ab = np.concatenate([Aout, Bout, nyq], axis=0).astype(BF16NP)  # [1025, 513]
    return dict(wall=wall, ab=ab, biasc=biasc)


_WAIT_EXEMPT = {
    "InstNoOp", "InstEventSemaphore", "InstUnconditionalBranch",
    "InstRegisterMove", "InstCall", "InstISA",
}


def _legalize_waits(nc, max_waits=1):
    """TRN2 instruction structs hold one sync-wait command; move extra waits
    onto same-engine nops inserted just before the instruction."""
    import bass_rust
    import concourse.mybir as mybir
    ctr = 0
    for fn in nc.m.functions:
        for blk in fn.blocks:
            new = []
            for inst in blk.instructions:
                if (type(inst).__name__ not in _WAIT_EXEMPT
                        and inst.sync_info is not None):
                    waits = list(inst.sync_info.on_wait)
                    if len(waits) > max_waits:
                        for w in waits[:-max_waits]:
                            nop = mybir.InstNoOp(
                                name=f"I-lglnop-{ctr}", ins=[], outs=[])
                            ctr += 1
                            nop.engine = inst.engine
                            nop.sync_info = bass_rust.SyncInfo(
                                on_wait=[w], on_update=[])
                            new.append(nop)
                        inst.sync_info = bass_rust.SyncInfo(
                            on_wait=waits[-max_waits:],
                            on_update=inst.sync_info.on_update)
                new.append(inst)
            blk.instructions = new


def _build(has_bias):
    import concourse.bass as bass
    import concourse.mybir as mybir
    import concourse.tile as tile
    F32, F8, BF16 = mybir.dt.float32, mybir.dt.float8e4, mybir.dt.bfloat16
    AT = mybir.AluOpType
    DR = mybir.MatmulPerfMode.DoubleRow

    nc = bass.Bass("TRN2", target_bir_lowering=False, debug=False,
                   num_devices=NCORES)
    htp_d = nc.dram_tensor("htp", [128, NPANEL * 8192], F8,
                           kind="ExternalInput")
    w_d = nc.dram_tensor("wall", [128, 6 * 8192], F8, kind="ExternalInput")
    ab_d = nc.dram_tensor("ab", [1025, 513], BF16, kind="ExternalInput")
    outp_d = nc.dram_tensor("outp", [CHUNK, D], BF16, kind="ExternalInput")
    mask_d = nc.dram_tensor("mask", [128, 64], F32, kind="ExternalInput")
    if has_bias:
        biasc_d = nc.dram_tensor("biasc", [128, 24], F32,
                                 kind="ExternalInput")
    res_d = nc.dram_tensor("res", [CHUNK, D], BF16, kind="ExternalOutput")
    cc_in = nc.dram_tensor("cc_in", [128, 8], F32)
    cc_out = nc.dram_tensor("cc_out", [NCORES * 128, 8], F32)

    with tile.TileContext(nc) as tc:
        with (
            tc.tile_pool(name="wpool", bufs=1) as wp,
            tc.tile_pool(name="htpool", bufs=1) as hp,
            tc.tile_pool(name="const", bufs=1) as cp,
            tc.tile_pool(name="fkv", bufs=2) as fkp,
            tc.tile_pool(name="fq3", bufs=4) as fqp,
            tc.tile_pool(name="tt", bufs=6) as ttp,
            tc.tile_pool(name="mem", bufs=1) as memp,
            tc.tile_pool(name="z", bufs=2) as zp,
            tc.tile_pool(name="io", bufs=6) as iop,
            tc.tile_pool(name="sv", bufs=5) as svp,
            tc.tile_pool(name="ps", bufs=2, space="PSUM") as psp,
        ):
            # DMA order is tuned so the PE can start ~7us in: hi halves of
            # the k/v weights and panel-0 ht first, then the lo halves.
            w = [wp.tile([128, 8192], F8, tag=f"w{m}", name=f"w{m}") for m in range(6)]
            htp = [hp.tile([128, 8192], F8, tag=f"ht{p}", name=f"ht{p}")
                   for p in range(NPANEL)]
            wvd = w_d.ap().rearrange("p (m dp hl f) -> p m dp hl f",
                                     m=6, dp=NDP, hl=2)
            htd = htp_d.ap().rearrange("p (pan dp hl x) -> p pan dp hl x",
                                       pan=NPANEL, dp=NDP, hl=2)
            wvs = [w[m][:].rearrange("p (dp hl f) -> p dp hl f",
                                     dp=NDP, hl=2) for m in range(6)]
            hts = [htp[p][:].rearrange("p (dp hl x) -> p dp hl x",
                                       dp=NDP, hl=2) for p in range(NPANEL)]

            def load_w(m, hl, pairs=((0, 8),)):
                for (a, b) in pairs:
                    nc.sync.dma_start(wvs[m][:, a:b, hl, :],
                                      wvd[:, m, a:b, hl, :])

            def load_ht(p, hl, pairs=((0, 8),)):
                for (a, b) in pairs:
                    nc.sync.dma_start(hts[p][:, a:b, hl, :],
                                      htd[:, p, a:b, hl, :])

            # first weight/data slices arrive pair-granular and interleaved
            # so the first matmul can start after two small transfers
            for (a, b) in ((0, 2), (2, 4), (4, 6), (6, 8)):
                load_ht(0, 0, ((a, b),))
                load_w(0, 0, ((a, b),))
            load_w(0, 1, ((0, 2), (2, 4), (4, 6), (6, 8)))
            load_ht(0, 1)
            for m in range(1, 4):
                load_w(m, 0)
                load_w(m, 1)
            load_w(4, 0)
            load_w(5, 0)
            load_w(4, 1)
            load_w(5, 1)
            for p in range(1, NPANEL):
                load_ht(p, 0)
                load_ht(p, 1)
            ab = [wp.tile([128, 513], BF16, tag=f"ab{i}", name=f"ab{i}")
                  for i in range(8)]
            for i in range(8):
                nc.sync.dma_start(ab[i][:], ab_d.ap()[i * 128:(i + 1) * 128, :])
            bny = wp.tile([1, 513], BF16, tag="bny", name="bny")
            nc.sync.dma_start(bny[:], ab_d.ap()[1024:1025, :])
            mask = cp.tile([128, 64], F32, tag="mask", name="mask")
            nc.sync.dma_start(mask[:], mask_d.ap())
            if has_bias:
                biasc = cp.tile([128, 24], F32, tag="biasc", name="biasc")
                nc.sync.dma_start(biasc[:], biasc_d.ap())

            wv, htv = wvs, hts

            def fwd_matmuls(pt, m, pan, ft, combos=3):
                # hh first (hi weights + hi data), then lh (lo weights),
                # then hl (lo data) — matches the DMA arrival order.
                # k/v (m<4) run 2 combos (full-W x hi-X): the dropped W@Xlo
                # term costs ~1e-2 rel err, inside the 2e-2 budget; q keeps
                # all 3 (its error hits Z unaveraged).
                ii = 0
                for (whl, xhl) in ((0, 0), (1, 0), (0, 1))[:combos]:
                    for a in range(NPAIR):
                        nc.tensor.matmul(
                            pt[:],
                            wv[m][:, 2 * a:2 * a + 2, whl,
                                  ft * 128:(ft + 1) * 128],
                            htv[pan][:, 2 * a:2 * a + 2, xhl, :],
                            start=(ii == 0), stop=(ii == combos * NPAIR - 1),
                            perf_mode=DR)
                        ii += 1

            mem = {}
            # ---- loop A: fk, fv, kv, local scan --------------------------
            for pan in range(NPANEL):
                for ft in range(FT):
                    ps = {}
                    for m, nm in enumerate(("kre", "kim", "vre", "vim")):
                        pt = psp.tile([128, PANEL], F32, tag=f"ps_{nm}", name=f"ps_{nm}")
                        fwd_matmuls(pt, m, pan, ft, combos=2)
                        ps[nm] = pt
                    s = {}
                    for m, nm in enumerate(("kre", "kim", "vre", "vim")):
                        t = fkp.tile([128, PANEL], BF16, tag=f"s_{nm}", name=f"s_{nm}")
                        nc.scalar.copy(t[:], ps[nm][:])
                        if has_bias:
                            c = m * 4 + ft
                            nc.vector.tensor_scalar_add(
                                t[:], t[:], biasc[:, c:c + 1])
                        s[nm] = t
                    # complex product folded into the scans:
                    # re: state = (state + t1) - t2, im: state = (state + t3) + t4
                    t1 = ttp.tile([128, PANEL], BF16, tag="tt", name="tt")
                    nc.vector.tensor_tensor(t1[:], s["kre"][:], s["vre"][:],
                                            op=AT.mult)
                    t2 = ttp.tile([128, PANEL], BF16, tag="tt", name="tt")
                    nc.vector.tensor_tensor(t2[:], s["kim"][:], s["vim"][:],
                                            op=AT.mult)
                    t3 = ttp.tile([128, PANEL], BF16, tag="tt", name="tt")
                    nc.vector.tensor_tensor(t3[:], s["kre"][:], s["vim"][:],
                                            op=AT.mult)
                    t4 = ttp.tile([128, PANEL], BF16, tag="tt", name="tt")
                    nc.vector.tensor_tensor(t4[:], s["kim"][:], s["vre"][:],
                                            op=AT.mult)

                    def init_of(ri, rows=slice(None)):
                        if pan == 0:
                            return 0.0
                        return mem[(pan - 1, ri, ft)][rows, PANEL - 1:PANEL]

                    mre = memp.tile([128, PANEL], BF16,
                                    tag=f"mem_re{ft}_{pan}",
                                    name=f"mem_re{ft}_{pan}")
                    nc.vector.tensor_tensor_scan(
                        mre[:], t1[:], t2[:], init_of("re"),
                        op0=AT.add, op1=AT.subtract)
                    mim = memp.tile([128, PANEL], BF16,
                                    tag=f"mem_im{ft}_{pan}",
                                    name=f"mem_im{ft}_{pan}")
                    nc.vector.tensor_tensor_scan(
                        mim[:], t3[:], t4[:], init_of("im"),
                        op0=AT.add, op1=AT.add)
                    if ft == 0:
                        # row 0 carries (DC, nyquist): pure real products
                        # (re += t1 only, im += t2 only); rescan that row.
                        nc.vector.tensor_tensor_scan(
                            mre[0:1, :], t1[0:1, :], t1[0:1, :],
                            init_of("re", slice(0, 1)),
                            op0=AT.add, op1=AT.bypass)
                        nc.vector.tensor_tensor_scan(
                            mim[0:1, :], t2[0:1, :], t2[0:1, :],
                            init_of("im", slice(0, 1)),
                            op0=AT.add, op1=AT.bypass)
                    mem[(pan, "re", ft)] = mre
                    mem[(pan, "im", ft)] = mim

            # ---- totals exchange (hidden under fq matmuls) ---------------
            tot = cp.tile([128, 8], F32, tag="tot", name="tot")
            for ft in range(FT):
                nc.gpsimd.tensor_copy(
                    tot[:, ft:ft + 1],
                    mem[(NPANEL - 1, "re", ft)][:, PANEL - 1:PANEL])
                nc.gpsimd.tensor_copy(
                    tot[:, 4 + ft:5 + ft],
                    mem[(NPANEL - 1, "im", ft)][:, PANEL - 1:PANEL])
            nc.sync.dma_start(cc_in.ap(), tot[:])
            # preload the residual into the output buffer (DRAM->DRAM) on
            # the same software-DGE queue the tail accumulates use, so the
            # final per-sub DMA is a single accumulate instead of two hops;
            # emitted here so it rides the post-loop-A DMA lull
            for r0 in range(0, CHUNK, PANEL):
                nc.gpsimd.dma_start(res_d.ap()[r0:r0 + PANEL, :],
                                    outp_d.ap()[r0:r0 + PANEL, :])
            nc.gpsimd.collective_compute(
                "AllGather", AT.bypass,
                replica_groups=[list(range(NCORES))],
                ins=[cc_in[:].opt()], outs=[cc_out[:].opt()])
            g = cp.tile([128, 64], F32, tag="g", name="g")
            nc.sync.dma_start(
                g[:].rearrange("p (c j) -> p c j", c=NCORES),
                cc_out.ap().rearrange("(c p) j -> p c j", c=NCORES))
            gm = cp.tile([128, 64], F32, tag="gm", name="gm")
            nc.vector.tensor_tensor(gm[:], g[:], mask[:], op=AT.mult)
            pref = cp.tile([128, 8], F32, tag="pref", name="pref")
            nc.vector.tensor_reduce(
                pref[:], gm[:].rearrange("p (c j) -> p j c", c=8),
                axis=mybir.AxisListType.X, op=AT.add)

            # ---- loop C: fq, prefix, Z, output matmul, residual ----------
            # fq blocks run two panels ahead of the Z/output blocks so the
            # PE keeps streaming fq matmuls while the AllGather completes.
            def fq_block(pan):
                sq = {}
                for ft in range(FT):
                    for m, nm in ((4, "qre"), (5, "qim")):
                        pt = psp.tile([128, PANEL], F32,
                                      tag=("ps_kre" if nm == "qre"
                                           else "ps_kim"),
                                      name=f"ps_{nm}")
                        fwd_matmuls(pt, m, pan, ft)
                        t = fqp.tile([128, PANEL], BF16, tag=f"s_{nm}{ft}",
                                     name=f"s_{nm}{ft}")
                        nc.scalar.activation(
                            t[:], pt[:], mybir.ActivationFunctionType.Copy,
                            scale=2.0 ** -19)
                        if has_bias:
                            c = m * 4 + ft
                            nc.vector.tensor_scalar_add(
                                t[:], t[:], biasc[:, c:c + 1])
                        sq[(nm, ft)] = t
                return sq

            def zprod_block(pan, sq):
                z = {}
                for ft in range(FT):
                    mre = mem[(pan, "re", ft)]
                    mim = mem[(pan, "im", ft)]
                    nc.vector.tensor_scalar_add(mre[:], mre[:],
                                                pref[:, ft:ft + 1])
                    nc.vector.tensor_scalar_add(mim[:], mim[:],
                                                pref[:, 4 + ft:5 + ft])
                    sqre, sqim = sq[("qre", ft)], sq[("qim", ft)]
                    for ri, (a, b_) in (("re", (sqre, sqim)),
                                        ("im", (sqim, sqre))):
                        neg = ri == "re"
                        t1 = ttp.tile([128, PANEL], BF16, tag="tt", name="tt")
                        nc.vector.tensor_tensor(t1[:], mre[:], a[:],
                                                op=AT.mult)
                        t2 = ttp.tile([128, PANEL], BF16, tag="tt", name="tt")
                        eng_m = (nc.gpsimd if ri == "im" and ft % 2 == 0
                                 else nc.vector)
                        eng_m.tensor_tensor(t2[:], mim[:], b_[:],
                                            op=AT.mult)
                        zt = zp.tile([128, PANEL], BF16, tag=f"z_{ri}{ft}",
                                     name=f"z_{ri}{ft}")
                        eng = nc.gpsimd if neg else nc.vector
                        eng.tensor_tensor(
                            zt[:], t1[:], t2[:],
                            op=(AT.subtract if neg else AT.add))
                        if ft == 0:
                            # row 0 carries (DC, nyq): plain real products
                            nc.vector.tensor_tensor(
                                zt[0:1, :], (mre if neg else mim)[0:1, :],
                                (sqre if neg else sqim)[0:1, :], op=AT.mult)
                        z[(ri, ft)] = zt
                return z

            def zout_mm(pan, z):
                for sub in range(4):
                    r0 = pan * PANEL + sub * 128
                    rs = iop.tile([128, D], BF16, tag="rs", name="rs")
                    s0, s1c = sub * 128, (sub + 1) * 128
                    # U = sum_ft zre.A1 (+ nyquist row, even, via K=1),
                    # V = sum zim.B1 (sin rows only). vals[0:512] = U + V,
                    # vals[512+j] = U[512-j] - V[512-j]; the d=512 edge
                    # column accumulates into V's (all-zero) column 0.
                    # Rotate over all four PSUM tag pairs (the fq tags are
                    # free once the zout phase runs) so four subs can be in
                    # flight before a combine has to retire.
                    tU, tV = (("ps_vre", "ps_vim") if sub % 2 == 0
                              else ("ps_kre", "ps_kim"))
                    psU = psp.tile([128, 512], F32, tag=tU, name="ps_U")
                    psV = psp.tile([128, 512], F32, tag=tV, name="ps_V")
                    for ft in range(FT):
                        nc.tensor.matmul(
                            psU[:], z[("re", ft)][:, s0:s1c],
                            ab[ft][:, 0:512], start=(ft == 0), stop=False)
                    nc.tensor.matmul(
                        psU[:], z[("im", 0)][0:1, s0:s1c],
                        bny[0:1, 0:512], start=False, stop=True)
                    for ft in range(FT):
                        nc.tensor.matmul(
                            psV[:], z[("im", ft)][:, s0:s1c],
                            ab[4 + ft][:, 0:512],
                            start=(ft == 0), stop=(ft == FT - 1))
                    for ft in range(FT):
                        nc.tensor.matmul(
                            psV[:, 0:1], z[("re", ft)][:, s0:s1c],
                            ab[ft][:, 512:513], start=(ft == 0), stop=False)
                    nc.tensor.matmul(
                        psV[:, 0:1], z[("im", 0)][0:1, s0:s1c],
                        bny[0:1, 512:513], start=False, stop=True)
                    # bf16 combine: both PSUM halves copied to SBUF by the
                    # Act engine so the DVE ops run at the 2-byte rate and
                    # the residual/output DMAs halve their traffic. Each
                    # 512-column half ships as soon as it is complete.
                    sU = svp.tile([128, 512], BF16, tag="sU", name="sU")
                    nc.scalar.copy(sU[:], psU[:])
                    sV = svp.tile([128, 512], BF16, tag="sV", name="sV")
                    nc.scalar.copy(sV[:], psV[:])
                    nc.vector.tensor_copy(rs[:, 0:1], sU[:, 0:1])
                    nc.vector.tensor_copy(rs[:, 512:513], sV[:, 0:1])
                    nc.vector.tensor_tensor(rs[:, 513:1024],
                                            sU[:, 511:0:-1],
                                            sV[:, 511:0:-1],
                                            op=AT.subtract)
                    nc.vector.tensor_tensor(rs[:, 1:512], sU[:, 1:512],
                                            sV[:, 1:512], op=AT.add)
                    # single accumulate-out into the preloaded res buffer
                    nc.gpsimd.dma_start(res_d.ap()[r0:r0 + 128, :], rs[:],
                                        accum_op=AT.add)

            # all fq blocks run first: ~40us of PE work that fully hides the
            # AllGather + prefix chain; z-product blocks stay one panel ahead
            # of the output matmuls so the PE never waits on the DVE.
            sqs = {p: fq_block(p) for p in range(NPANEL)}
            zps = {0: zprod_block(0, sqs[0]), 1: zprod_block(1, sqs[1])}
            for pan in range(NPANEL):
                if pan + 2 < NPANEL:
                    zps[pan + 2] = zprod_block(pan + 2, sqs[pan + 2])
                zout_mm(pan, zps.pop(pan))

    _legalize_waits(nc)
    return nc


def _program(has_bias=False):
    key = ("merged", has_bias)
    if key not in _cache:
        _cache[key] = _build(has_bias)
    return _cache[key]


def kernel(output, hidden_states, Wq, bq, Wk, bk, Wv, bv, gate, _trace=False):
    from concourse import bass_utils

    output = np.asarray(output, dtype=np.float32)
    hidden = np.asarray(hidden_states, dtype=np.float32)
    cst = _host_constants(
        np.asarray(Wq, np.float32), np.asarray(bq, np.float32),
        np.asarray(Wk, np.float32), np.asarray(bk, np.float32),
        np.asarray(Wv, np.float32), np.asarray(bv, np.float32),
        np.asarray(gate, np.float32))
    has_bias = bool(np.any(cst["biasc"]))
    nc = _program(has_bias)

    chunks = [(c // 4, c % 4) for c in range(NCORES)]
    shared = {"wall": cst["wall"], "ab": cst["ab"]}
    if has_bias:
        bc = np.zeros((128, 24), np.float32)
        for m in range(6):
            bc[:, m * 4:(m + 1) * 4] = cst["biasc"][m].reshape(4, 128).T
        shared["biasc"] = bc

    in_maps = []
    for c, (b, j) in enumerate(chunks):
        im = dict(shared)
        ht = np.ascontiguousarray(
            hidden[b, j * CHUNK:(j + 1) * CHUNK, :].T)
        im["htp"] = _pack_ht(ht)
        im["outp"] = output[b, j * CHUNK:(j + 1) * CHUNK, :].astype(BF16NP)
        mask = np.zeros((128, 64), np.float32)
        for c2, (b2, j2) in enumerate(chunks):
            if b2 == b and j2 < j:
                mask[:, c2 * 8:(c2 + 1) * 8] = 1.0
        im["mask"] = mask
        in_maps.append(im)

    res = bass_utils.run_bass_kernel_spmd(
        nc, in_maps, core_ids=list(range(NCORES)), trace=_trace)

    out = np.empty((B, S, D), dtype=np.float32)
    for c, (b, j) in enumerate(chunks):
        out[b, j * CHUNK:(j + 1) * CHUNK, :] = res.results[c]["res"].astype(
            np.float32)
    if _trace:
        kernel._last = res
    return out

